# revision 1
# baseline (speedup 1.0000x reference)
"""FlowFeature (bilinear warp + local correlation) Trainium2 kernel.

Strategy (per core; 8 cores = 4 batches x 2 row-halves):
  * Host computes exact bilinear-warp weights from the flow and encodes the
    warp as sparse column-mix matrices T (the warp is a banded linear map on
    each row-triple of cp_r).
  * Device stage 1 (PE): warped rows  W_i[c, m] = sum_{j'} R^T[j', c] * T[j', m]
    via matmuls with the transposed cp_r rows as stationary operands.
    Accumulation over the 3 source rows happens in PSUM.
  * Device stage 2 (PE): correlation gram blocks  corr[x, dy, m] for 32-wide
    x-groups (col-tiled matmuls, contraction over channels), giving a
    semi-compact band of 9dy x 40m per output pixel.
  * DVE/ACT evacuate PSUM (cast to bf16); DMA ships the semi-compact band to
    HBM; the host extracts the 9x9 displacement band (a cheap strided view).

Self-contained: hardcodes shapes/sharding for B,H,W,C = 4,128,256,128, md=4.
"""
import os
from contextlib import ExitStack

import numpy as np
import ml_dtypes

import concourse.bass as bass
import concourse.tile as tile
from concourse import bacc, mybir
from concourse.bass_utils import run_bass_kernel_spmd

B, H, W, C = 4, 128, 256, 128
FLOW_SCALE = 0.05
MD = 4
ND = 9            # displacements per axis
NCORES = 8
YH = 64           # output rows per core
R = 74            # source-row slots per core (r = y0-5+rloc, rloc in [0,74))
NSLOT = 72        # warp-row slots (i = y0-4+s, s in [0,72))
WB = 264          # warp row buffer cols (j in [-4, 260))
NG = 4            # col-groups per 128-x block (32 wide each)
WIN = 40          # corr moving window per group
BAND = ND * WIN   # 360 semi-compact band els per pixel

# warp j'-slabs (source column ranges) and the m-ranges they cover
SLABS = [(0, 128), (126, 254), (252, 256)]
MQ = [(0, 127), (127, 253), (253, 256)]

BF16 = ml_dtypes.bfloat16


# ----------------------------------------------------------------------------
# host side: weights + layouts
# ----------------------------------------------------------------------------

def _interp_weights(flow_bhw2):
    """Per-pixel 3-tap weights in y and x, exactly matching the reference
    bilinear warp (incl. border clamps). Returns wy, wx: [3, B, H, W] f32,
    tap index t in {0,1,2} <-> offset u = t-1."""
    f = flow_bhw2.astype(np.float32) * np.float32(FLOW_SCALE)
    fy, fx = f[..., 0], f[..., 1]
    gy = np.arange(H, dtype=np.float32)[None, :, None]
    gx = np.arange(W, dtype=np.float32)[None, None, :]
    qy = gy - fy
    qx = gx - fx
    y0 = np.clip(np.floor(qy), 0, H - 2).astype(np.int32)
    x0 = np.clip(np.floor(qx), 0, W - 2).astype(np.int32)
    ay = np.clip(qy - y0.astype(np.float32), 0.0, 1.0)
    ax = np.clip(qx - x0.astype(np.float32), 0.0, 1.0)
    sy = y0 - gy.astype(np.int32)   # in {-1, 0}
    sx = x0 - gx.astype(np.int32)
    wy = np.zeros((3,) + fy.shape, np.float32)
    wx = np.zeros((3,) + fx.shape, np.float32)
    for u in (-1, 0, 1):
        wy[u + 1] = (1.0 - ay) * (sy == u) + ay * ((sy + 1) == u)
        wx[u + 1] = (1.0 - ax) * (sx == u) + ax * ((sx + 1) == u)
    return wy, wx


def _host_prep(cp_l, cp_r, up_flowq):
    wy, wx = _interp_weights(up_flowq)
    in_maps = []
    for core in range(NCORES):
        b, half = divmod(core, 2)
        y0 = YH * half
        r_base = y0 - 5

        # cp_l rows, [c, yl, x], pre-scaled by 1/C
        lt = (cp_l[b, y0:y0 + YH].astype(np.float32) / np.float32(C))
        lt = np.ascontiguousarray(lt.transpose(2, 0, 1)).reshape(C, YH * W)

        # cp_r rows with zero padding outside [0, H)
        rt_all = np.zeros((R, W, C), np.float32)
        lo = max(0, r_base)
        hi = min(H, r_base + R)
        rt_all[lo - r_base:hi - r_base] = cp_r[b, lo:hi]
        slabs = []
        for (j0, j1) in SLABS:
            s = np.ascontiguousarray(rt_all[:, j0:j1].transpose(1, 0, 2))
            slabs.append(s.reshape(j1 - j0, R * C).astype(BF16))
        rta, rtb, rtc = slabs

        # warp matrices T[rloc, jloc, o, m]
        T = np.zeros((R, 3, 128, W), np.float32)   # [rloc, o, jloc, m]
        r_arr = r_base + np.arange(R)
        for o in (-1, 0, 1):
            i_arr = r_arr + o
            rsel = np.where((r_arr >= 0) & (r_arr < H)
                            & (i_arr >= 0) & (i_arr < H))[0]
            if len(rsel) == 0:
                continue
            ii = i_arr[rsel]
            wyo = wy[(-o) + 1][b][ii]        # [nr, W]
            for q, ((j0, j1), (m0, m1)) in enumerate(zip(SLABS, MQ)):
                for v in (-1, 0, 1):
                    m = np.arange(m0, m1)
                    jloc = m + v - j0
                    ok = (jloc >= 0) & (jloc < (j1 - j0))
                    m = m[ok]
                    jl = jloc[ok]
                    if len(m) == 0:
                        continue
                    vals = wyo[:, m] * wx[v + 1][b][ii][:, m]
                    T[rsel[:, None], o + 1, jl[None, :], m[None, :]] = vals
        tmat = np.ascontiguousarray(T.transpose(0, 2, 1, 3))  # [R, jloc, o, m]
        tmat = tmat.reshape(R, 128, 3 * W).astype(BF16)

        in_maps.append({
            "lt": lt.astype(BF16),
            "rta": rta, "rtb": rtb, "rtc": rtc,
            "tmat": tmat,
        })
    return in_maps


def _unshard(results):
    out = np.zeros((B, H, W, ND * ND), np.float32)
    for core in range(NCORES):
        b, half = divmod(core, 2)
        y0 = YH * half
        semi = results[core]["out"].astype(np.float32)   # [64, 2, 128, 360]
        v = semi.reshape(YH, 2, NG, 32, ND, WIN)
        s = v.strides
        band = np.lib.stride_tricks.as_strided(
            v, shape=(YH, 2, NG, 32, ND, ND),
            strides=(s[0], s[1], s[2], s[3] + s[5], s[4], s[5]))
        # band[yl, xb, gl, p32, dy, dx] -> out[y, x, dy*9+dx]
        band = band.reshape(YH, W, ND * ND)
        out[b, y0:y0 + YH] = band
    return out


# ----------------------------------------------------------------------------
# device kernel
# ----------------------------------------------------------------------------

def _emit(tc, nc, io):
    bf = mybir.dt.bfloat16
    f32 = mybir.dt.float32
    lt_d, rta_d, rtb_d, rtc_d, tmat_d, out_d = (
        io["lt"], io["rta"], io["rtb"], io["rtc"], io["tmat"], io["out"])

    with ExitStack() as ctx:
        const = ctx.enter_context(tc.tile_pool(name="const", bufs=1))
        tpool = ctx.enter_context(tc.tile_pool(name="tring", bufs=8))
        wpsum = ctx.enter_context(tc.tile_pool(name="wpsum", bufs=4, space="PSUM"))
        cpsum = ctx.enter_context(tc.tile_pool(name="cpsum", bufs=3, space="PSUM"))
        stg = ctx.enter_context(tc.tile_pool(name="stg", bufs=2))

        # resident inputs
        lt = const.tile([C, YH * W], bf, tag="lt")
        nc.sync.dma_start(lt[:], lt_d.ap()[:])
        rta = const.tile([128, R * C], bf, tag="rta")
        nc.sync.dma_start(rta[:], rta_d.ap()[:])
        rtb = const.tile([128, R * C], bf, tag="rtb")
        nc.sync.dma_start(rtb[:], rtb_d.ap()[:])
        rtc = const.tile([4, R * C], bf, tag="rtc")
        nc.sync.dma_start(rtc[:], rtc_d.ap()[:])
        slab_tiles = [rta, rtb, rtc]

        # warp row buffer [c, NSLOT x WB]
        wbuf = const.tile([C, NSLOT * WB], bf, tag="wbuf")
        wb_t, wb_off = wbuf[:].tensor, wbuf[:].offset
        # zero the 4-col pads on each side of every slot
        for base in (0, WB - 4):
            pad = bass.AP(wb_t, wb_off + base, [[NSLOT * WB, C], [WB, NSLOT], [1, 4]])
            nc.vector.memset(pad, 0.0)

        slot_psum = {}
        stag = None
        for vr in range(R + 1):
            # ---- stage 1: warp matmuls for source slot rloc = vr
            if vr < R:
                rloc = vr
                tm = tpool.tile([128, 3 * W], bf, tag="tm")
                nc.sync.dma_start(tm[:], tmat_d.ap()[rloc])
                for o in (-1, 0, 1):
                    s = rloc + o - 1
                    if not (0 <= s < NSLOT):
                        continue
                    if s not in slot_psum:
                        ps = wpsum.tile([C, 512], f32, tag="wp", name=f"wp{s}")
                        slot_psum[s] = ps
                    ps = slot_psum[s]
                    first = (rloc == s)          # first contributor (o=+1)
                    last = (rloc == s + 2)       # last contributor (o=-1)
                    for q, ((j0, j1), (m0, m1)) in enumerate(zip(SLABS, MQ)):
                        span = j1 - j0
                        st = slab_tiles[q][:]
                        stat = bass.AP(
                            st.tensor, st.offset + rloc * C,
                            [[R * C, span], [1, C]])
                        mov = tm[0:span, (o + 1) * W + m0:(o + 1) * W + m1]
                        nc.tensor.matmul(
                            ps[:, m0:m1], stat, mov,
                            start=(first and q == 0), stop=(last and q == 2),
                            skip_group_check=True)

            # ---- stage 1b: evacuate completed warp slots (s done after r=s+2)
            done = []
            if vr >= 2:
                done.append(vr - 2)
            if vr == R:
                done.append(NSLOT - 1)  # s=71 completes at rloc=73=R-1
            for s in done:
                if s not in slot_psum or s >= NSLOT:
                    continue
                ps = slot_psum.pop(s)
                dst = bass.AP(wb_t, wb_off + s * WB + 4, [[NSLOT * WB, C], [1, W]])
                if s % 2 == 0:
                    nc.vector.tensor_copy(dst, ps[:, 0:W])
                else:
                    nc.scalar.copy(dst, ps[:, 0:W])

            # ---- stage 2: correlation for output row yl (ready at vr = yl+10)
            yl = vr - 10
            if not (0 <= yl < YH):
                continue
            if yl % 8 == 0:
                stag = stg.tile([128, 8 * 2 * BAND], bf, tag="stg")
            for xb in range(2):
                cp = cpsum.tile([128, 512], f32, tag="cp", name=f"cp{yl}_{xb}")
                for gl in range(NG):
                    stat = lt[:, yl * W + 128 * xb + 32 * gl:
                              yl * W + 128 * xb + 32 * gl + 32]
                    for t in range(3):
                        mov = bass.AP(
                            wb_t,
                            wb_off + (yl + 3 * t) * WB + 32 * (NG * xb + gl),
                            [[NSLOT * WB, C], [WB, 3], [1, WIN]])
                        nc.tensor.matmul(
                            cp[32 * gl:32 * gl + 32,
                               3 * t * WIN:(3 * t + 3) * WIN],
                            stat, mov,
                            start=(t == 0), stop=(t == 2),
                            tile_position=(0, 32 * gl),
                            skip_group_check=True)
                dst = stag[:, ((yl % 8) * 2 + xb) * BAND:
                           ((yl % 8) * 2 + xb + 1) * BAND]
                if (2 * yl + xb) % 2 == 0:
                    nc.scalar.copy(dst, cp[:, 0:BAND])
                else:
                    nc.vector.tensor_copy(dst, cp[:, 0:BAND])
            if yl % 8 == 7:
                blk = yl // 8
                dst = bass.AP(
                    out_d.ap().tensor, blk * 8 * 2 * 128 * BAND,
                    [[BAND, 128], [2 * 128 * BAND, 8], [128 * BAND, 2], [1, BAND]])
                nc.sync.dma_start(dst, stag[:])


_NC_CACHE = {}


def _build_nc():
    if "nc" in _NC_CACHE:
        return _NC_CACHE["nc"]
    bf = mybir.dt.bfloat16
    nc = bacc.Bacc("TRN2", target_bir_lowering=False, debug=False,
                   num_devices=NCORES)
    io = {
        "lt": nc.dram_tensor("lt", [C, YH * W], bf, kind="ExternalInput"),
        "rta": nc.dram_tensor("rta", [128, R * C], bf, kind="ExternalInput"),
        "rtb": nc.dram_tensor("rtb", [128, R * C], bf, kind="ExternalInput"),
        "rtc": nc.dram_tensor("rtc", [4, R * C], bf, kind="ExternalInput"),
        "tmat": nc.dram_tensor("tmat", [R, 128, 3 * W], bf, kind="ExternalInput"),
        "out": nc.dram_tensor("out", [YH, 2, 128, BAND], bf,
                              kind="ExternalOutput"),
    }
    with tile.TileContext(nc) as tc:
        _emit(tc, nc, io)
    nc.compile()
    _NC_CACHE["nc"] = nc
    return nc


def kernel(cp_l, cp_r, up_flowq):
    cp_l = np.asarray(cp_l)
    cp_r = np.asarray(cp_r)
    up_flowq = np.asarray(up_flowq)
    in_maps = _host_prep(cp_l, cp_r, up_flowq)
    nc = _build_nc()
    res = run_bass_kernel_spmd(nc, in_maps, core_ids=list(range(NCORES)))
    return _unshard(res.results)

